# revision 18
# baseline (speedup 1.0000x reference)
"""LSTM caption decoder on 8 TRN2 NeuronCores — hidden-sharded.

Problem: 24-step LSTMCell (B=128, E=512, H=1024) + vocab projection (V=12000).

Strategy (vs the replicated-LSTM baseline): shard the LSTM hidden dim 8-way.
Core j owns hidden columns j*128..(j+1)*128, i.e. 512 of the 4096 gate
columns (128 each of i,g,f,o). Per step each core computes only its gate
slice (contraction over the FULL h), updates its c/h column block, and
all-gathers its h_t chunk [128, B] to the other cores with an ncfw
collective AllGather (SBUF -> HBM -> AllGather -> HBM -> SBUF), bounced
through a triple-buffered recv ring. The vocab projection stays
column-sharded (1500 cols/core) and is interleaved into the step loop so
the PE computes FC_{t-1} while the h_t all-gather is in flight.

(A faster SBUF->SBUF remote_dma_broadcast path exists and validates in the
8-core interpreter, but issuing more than one data broadcast crashes the
exec unit under this runtime — see kernel_rdma.py; sem-only broadcasts
repeat fine. The ncfw collective is the supported path; its ~9us Mesh
latency + ~25us one-time barrier dominate the remaining runtime.)

Layouts: gates are computed TRANSPOSED — out[gate_cols(128), B] with the
weight block stationary — so h_t is produced directly in [cols, B] chunk
layout and no transposes are needed anywhere. Gate bias rides the ACT op
(per-partition bias in transposed layout).

Per-core PE work/step: 48 gate matmuls (12 K-chunks x 4 col-chunks, 128
moving rows) + 24 FC matmuls (8 K-chunks x 3 vocab chunks, 500 rows)
= 6144 + 12000 cycles ~ 7.6us at 2.4GHz. Measured step period ~24.5us
(AG-latency bound): 635us total vs 651us for the replicated baseline.

All matmul inputs bf16 (fp32 PSUM accumulation); c state fp32; logits
stored bf16 (adds ~0.1% rms; tolerance is 2e-2). End-to-end rel err vs
the fp32 reference: 2.96e-3.
"""

import sys

if "/opt/trn_rl_repo" not in sys.path:
    sys.path.insert(0, "/opt/trn_rl_repo")

import numpy as np
import ml_dtypes

import concourse.bass as bass
import concourse.bacc as bacc
import concourse.tile as tile
from concourse import mybir
from concourse.bass import ds
from concourse.bass_utils import run_bass_kernel_spmd

B = 128
T = 24
E = 512
H = 1024
V = 12000
NCORES = 8
VS = V // NCORES          # 1500 vocab cols per core
KX = E // 128             # 4 contraction chunks for x-part
KH = H // 128             # 8 contraction chunks for h-part
NVC = 3                   # vocab chunks per core (3 x 500)
VC = VS // NVC            # 500
NSLOT = 3                 # recv ring depth for h chunks
TRIG_WAITS = True         # debug toggle
SKIP_COMMS = False        # debug toggle: no broadcasts/triggers/waits

F32 = mybir.dt.float32
BF16 = mybir.dt.bfloat16
AF = mybir.ActivationFunctionType
POOL = mybir.EngineType.Pool

# gate col-chunk order [i, g, f, o] -> activation per chunk
_CC_FUNC = [AF.Sigmoid, AF.Tanh, AF.Sigmoid, AF.Sigmoid]


def build_nc(for_sim: bool = False, nsteps: int = T - 1):
    """nsteps < T-1 builds a truncated kernel (debug bisection only)."""
    nc = bacc.Bacc("TRN2", target_bir_lowering=False, debug=False,
                   num_devices=NCORES)

    wih_d = nc.dram_tensor("wih", [128, KX, 4, 128], BF16, kind="ExternalInput").ap()
    whh_d = nc.dram_tensor("whh", [128, KH, 4, 128], BF16, kind="ExternalInput").ap()
    bg_d = nc.dram_tensor("bg", [128, 4], F32, kind="ExternalInput").ap()
    wfc_d = nc.dram_tensor("wfc", [128, KH, VS], BF16, kind="ExternalInput").ap()
    bfc_d = nc.dram_tensor("bfc", [128, VS], F32, kind="ExternalInput").ap()
    xt_d = nc.dram_tensor("xt", [T - 1, 128, KX, B], BF16, kind="ExternalInput").ap()
    ht0_d = nc.dram_tensor("ht0", [128, KH, B], BF16, kind="ExternalInput").ap()
    c0_d = nc.dram_tensor("c0", [128, B], F32, kind="ExternalInput").ap()
    out_d = nc.dram_tensor("logits", [T, B, VS], BF16, kind="ExternalOutput").ap()
    hch_d = nc.dram_tensor("hch", [2, 128, B], BF16, kind="Internal").ap()
    hall_d = nc.dram_tensor("hall", [2, KH, 128, B], BF16, kind="Internal").ap()

    # (instruction, sem, value) to inject after Tile scheduling
    pending_waits = []

    with tile.TileContext(nc) as tc:
        with (
            tc.tile_pool(name="weights", bufs=1) as wpool,
            tc.tile_pool(name="xin", bufs=3) as xpool,
            tc.tile_pool(name="gact", bufs=2) as gpool,
            tc.tile_pool(name="state", bufs=1) as spool,
            tc.tile_pool(name="hbuf", bufs=1) as hpool,
            tc.tile_pool(name="lout", bufs=4) as lpool,
            tc.tile_pool(name="pg", bufs=4, space="PSUM") as pgpool,
            tc.tile_pool(name="pf", bufs=3, space="PSUM") as pfpool,
        ):

            wih = wpool.tile([128, KX, 4, 128], BF16)
            whh = wpool.tile([128, KH, 4, 128], BF16)
            bg = wpool.tile([128, 4], F32)
            wfc = wpool.tile([128, KH, VS], BF16)
            bfc = wpool.tile([128, VS], F32)
            ht0 = hpool.tile([128, KH, B], BF16)
            c = spool.tile([128, B], F32)
            # recv ring: slot s chunk k (from core k) at recv[s][:, k, :]
            recv = [hpool.tile([128, KH, B], BF16, name=f"recv{s_}")
                    for s_ in range(NSLOT)]
            hsend = [hpool.tile([128, B], BF16, name=f"hsend{s_}")
                    for s_ in range(2)]
            tanh_c = spool.tile([128, B], F32)
            ig = spool.tile([128, B], F32)

            # prologue loads, consumer-ordered (sync queue): step-1 gate
            # deps first, then FC weights per vocab chunk.
            nc.sync.dma_start(ht0[:], ht0_d[:])
            nc.sync.dma_start(c[:], c0_d[:])
            xt1 = xpool.tile([128, KX, B], BF16, tag="xt")
            nc.sync.dma_start(xt1[:], xt_d[0])
            nc.sync.dma_start(wih[:], wih_d[:])
            nc.sync.dma_start(bg[:], bg_d[:])
            nc.sync.dma_start(whh[:], whh_d[:])
            for v in range(NVC):
                vsl = slice(v * VC, (v + 1) * VC)
                nc.sync.dma_start(wfc[:, :, vsl], wfc_d[:, :, vsl])
            nc.sync.dma_start(bfc[:], bfc_d[:])


            def hsrc(t):
                if t == 0:
                    return [ht0[:, k, :] for k in range(KH)]
                r = recv[t % NSLOT]
                return [r[:, k, :] for k in range(KH)]

            def emit_fc(t):
                src = hsrc(t)
                first = None
                for v in range(NVC):
                    vsl = slice(v * VC, (v + 1) * VC)
                    pf = pfpool.tile([B, VC], F32, tag="pf")
                    for k in range(KH):
                        mm = nc.tensor.matmul(
                            pf[:], src[k], wfc[:, k, vsl],
                            start=(k == 0), stop=(k == KH - 1),
                        )
                        if first is None:
                            first = mm
                    lo = lpool.tile([B, VC], BF16, tag="lo")
                    nc.vector.tensor_add(lo[:], pf[:], bfc[:, vsl])
                    nc.scalar.dma_start(out_d[t, :, vsl], lo[:])
                return first

            for t in range(1, nsteps + 1):
                if t == 1:
                    xt = xt1
                else:
                    xt = xpool.tile([128, KX, B], BF16, tag="xt")
                    nc.scalar.dma_start(xt[:], xt_d[t - 1])
                src_prev = hsrc(t - 1)

                # ---- gates (transposed): 4 col-chunks [i, g, f, o] ----
                # x-parts first (no cross-core dep: PE busy while h arrives)
                pgs = []
                for cc in range(4):
                    pg = pgpool.tile([128, 512], F32, tag="pg")
                    pgs.append(pg)
                    for k in range(KX):
                        nc.tensor.matmul(
                            pg[:, 0:B], wih[:, k, cc, :], xt[:, k, :],
                            start=(k == 0), stop=False,
                        )
                gact = []
                for cc in range(4):
                    pg = pgs[cc]
                    for k in range(KH):
                        mm = nc.tensor.matmul(
                            pg[:, 0:B], whh[:, k, cc, :], src_prev[k],
                            start=False, stop=(k == KH - 1),
                        )
                    a = gpool.tile([128, B], F32, tag=f"g{cc}")
                    nc.scalar.activation(a[:], pg[:, 0:B], _CC_FUNC[cc],
                                         bias=bg[:, cc:cc + 1])
                    gact.append(a)
                    if cc == 1:
                        nc.vector.tensor_mul(ig[:], gact[0][:], gact[1][:])
                    elif cc == 2:
                        nc.vector.tensor_mul(c[:], c[:], gact[2][:])
                        nc.vector.tensor_add(c[:], c[:], ig[:])
                        nc.scalar.activation(tanh_c[:], c[:], AF.Tanh)
                    elif cc == 3:
                        hs = hsend[t % 2]
                        nc.vector.tensor_mul(hs[:], gact[3][:], tanh_c[:])

                # ---- all-gather h_t via ncfw collective (HBM bounce) ----
                par = t % 2
                nc.sync.dma_start(hch_d[par], hsend[t % 2][:])
                nc.gpsimd.collective_compute(
                    "AllGather", mybir.AluOpType.bypass,
                    replica_groups=[list(range(NCORES))],
                    ins=[hch_d[par]], outs=[hall_d[par]],
                )
                r = recv[t % NSLOT]
                for k in range(KH):
                    nc.sync.dma_start(r[:, k, :], hall_d[par, k])

                # ---- FC of previous step: PE work overlapping the bcast ----
                emit_fc(t - 1)

            emit_fc(nsteps)

    if for_sim:
        # interp needs concrete registers + library loads, not ISA packing
        nc.dce_regs()
        nc.alloc_regs()
        nc.insert_library_loads()
        nc.insert_act_table_loads()
    else:
        nc.compile()
    return nc


_NC_CACHE = None


def _get_nc():
    global _NC_CACHE
    if _NC_CACHE is None:
        _NC_CACHE = build_nc()
    return _NC_CACHE


def _prep_inputs(encoder_output, captions, embed_table, W_ih, W_hh, b_ih, b_hh,
                 W_fc, b_fc):
    bf = ml_dtypes.bfloat16
    enc = np.asarray(encoder_output, np.float32)
    cap = np.asarray(captions).astype(np.int64)
    emb = np.asarray(embed_table, np.float32)
    W_ih = np.asarray(W_ih, np.float32)
    W_hh = np.asarray(W_hh, np.float32)
    W_fc = np.asarray(W_fc, np.float32)
    bgs = np.asarray(b_ih, np.float32) + np.asarray(b_hh, np.float32)
    b_fc = np.asarray(b_fc, np.float32)

    X = np.empty((T, B, E), np.float32)
    X[0] = enc
    X[1:] = emb[cap[:, : T - 1]].transpose(1, 0, 2)
    # xt[t,p,k,b] = X[t+1,b,k*128+p], steps 1..23
    xt = np.ascontiguousarray(
        X[1:].reshape(T - 1, B, KX, 128).transpose(0, 3, 2, 1)).astype(bf)

    # step 0 on host, fp32 (h_prev = c_prev = 0)
    gates0 = enc @ W_ih.T + bgs
    i0, f0, g0, o0 = np.split(gates0, 4, axis=-1)
    sig = lambda z: 1.0 / (1.0 + np.exp(-z))
    c0 = sig(i0) * np.tanh(g0)
    h0 = sig(o0) * np.tanh(c0)
    ht0 = np.ascontiguousarray(
        h0.T.reshape(KH, 128, B).transpose(1, 0, 2)).astype(bf)

    common = {"xt": xt, "ht0": ht0}
    in_maps = []
    for ci in range(NCORES):
        r = np.r_[ci * 128:(ci + 1) * 128]
        sel = np.concatenate([r, 2048 + r, 1024 + r, 3072 + r])  # [i,g,f,o]
        wih = np.ascontiguousarray(
            W_ih[sel].reshape(4, 128, KX, 128).transpose(3, 2, 0, 1)).astype(bf)
        whh = np.ascontiguousarray(
            W_hh[sel].reshape(4, 128, KH, 128).transpose(3, 2, 0, 1)).astype(bf)
        bg = np.ascontiguousarray(bgs[sel].reshape(4, 128).T)
        c0j = np.ascontiguousarray(c0[:, ci * 128:(ci + 1) * 128].T)
        vsl = slice(ci * VS, (ci + 1) * VS)
        wfc = np.ascontiguousarray(
            W_fc[vsl].reshape(VS, KH, 128).transpose(2, 1, 0)).astype(bf)
        bfc = np.ascontiguousarray(np.broadcast_to(b_fc[vsl], (128, VS)))
        in_maps.append({**common, "wih": wih, "whh": whh, "bg": bg,
                        "c0": c0j, "wfc": wfc, "bfc": bfc})
    return in_maps


def run_on_device(in_maps, trace=False, **kw):
    nc = _get_nc()
    return run_bass_kernel_spmd(
        nc, in_maps, list(range(NCORES)), trace=trace, **kw)


def _assemble(res):
    shards = [np.asarray(res.results[ci]["logits"]).astype(np.float32)
              for ci in range(NCORES)]
    full = np.concatenate(shards, axis=-1)  # [T, B, V]
    return np.ascontiguousarray(full.transpose(1, 0, 2))  # [B, T, V]


def kernel(encoder_output, captions, embed_table, W_ih, W_hh, b_ih, b_hh,
           W_fc, b_fc):
    in_maps = _prep_inputs(encoder_output, captions, embed_table,
                           W_ih, W_hh, b_ih, b_hh, W_fc, b_fc)
    res = run_on_device(in_maps)
    return _assemble(res)


# revision 19
# speedup vs baseline: 1.0310x; 1.0310x over previous
"""LSTM caption decoder on 8 TRN2 NeuronCores — hidden-sharded.

Problem: 24-step LSTMCell (B=128, E=512, H=1024) + vocab projection (V=12000).

Strategy (vs the replicated-LSTM baseline): shard the LSTM hidden dim 8-way.
Core j owns hidden columns j*128..(j+1)*128, i.e. 512 of the 4096 gate
columns (128 each of i,g,f,o). Per step each core computes only its gate
slice (contraction over the FULL h), updates its c/h column block, and
all-gathers its h_t chunk [128, B] to the other cores with an ncfw
collective AllGather (SBUF -> HBM -> AllGather -> HBM -> SBUF), bounced
through a triple-buffered recv ring. The vocab projection stays
column-sharded (1500 cols/core) and is interleaved into the step loop so
the PE computes FC_{t-1} while the h_t all-gather is in flight.

(A faster SBUF->SBUF remote_dma_broadcast path exists and validates in the
8-core interpreter, but issuing more than one data broadcast crashes the
exec unit under this runtime — see kernel_rdma.py; sem-only broadcasts
repeat fine. The ncfw collective is the supported path; its ~9us Mesh
latency + ~25us one-time barrier dominate the remaining runtime.)

Layouts: gates are computed TRANSPOSED — out[gate_cols(128), B] with the
weight block stationary — so h_t is produced directly in [cols, B] chunk
layout and no transposes are needed anywhere. Gate bias rides the ACT op
(per-partition bias in transposed layout).

Per-core PE work/step: 48 gate matmuls (12 K-chunks x 4 col-chunks, 128
moving rows) + 24 FC matmuls (8 K-chunks x 3 vocab chunks, 500 rows)
= 6144 + 12000 cycles ~ 7.6us at 2.4GHz. Measured step period ~24.5us
(AG-latency bound): 635us total vs 651us for the replicated baseline.

All matmul inputs bf16 (fp32 PSUM accumulation); c state fp32; logits
stored bf16 (adds ~0.1% rms; tolerance is 2e-2). End-to-end rel err vs
the fp32 reference: 2.96e-3.
"""

import sys

if "/opt/trn_rl_repo" not in sys.path:
    sys.path.insert(0, "/opt/trn_rl_repo")

import numpy as np
import ml_dtypes

import concourse.bass as bass
import concourse.bacc as bacc
import concourse.tile as tile
from concourse import mybir
from concourse.bass import ds
from concourse.bass_utils import run_bass_kernel_spmd

B = 128
T = 24
E = 512
H = 1024
V = 12000
NCORES = 8
VS = V // NCORES          # 1500 vocab cols per core
KX = E // 128             # 4 contraction chunks for x-part
KH = H // 128             # 8 contraction chunks for h-part
NVC = 3                   # vocab chunks per core (3 x 500)
VC = VS // NVC            # 500
NSLOT = 3                 # recv ring depth for h chunks
TRIG_WAITS = True         # debug toggle
SKIP_COMMS = False        # debug toggle: no broadcasts/triggers/waits

F32 = mybir.dt.float32
BF16 = mybir.dt.bfloat16
AF = mybir.ActivationFunctionType
POOL = mybir.EngineType.Pool

# gate col-chunk order [i, g, f, o] -> activation per chunk
_CC_FUNC = [AF.Sigmoid, AF.Tanh, AF.Sigmoid, AF.Sigmoid]


def build_nc(for_sim: bool = False, nsteps: int = T - 1):
    """nsteps < T-1 builds a truncated kernel (debug bisection only)."""
    nc = bacc.Bacc("TRN2", target_bir_lowering=False, debug=False,
                   num_devices=NCORES)

    wih_d = nc.dram_tensor("wih", [128, KX, 4, 128], BF16, kind="ExternalInput").ap()
    whh_d = nc.dram_tensor("whh", [128, KH, 4, 128], BF16, kind="ExternalInput").ap()
    bg_d = nc.dram_tensor("bg", [128, 4], F32, kind="ExternalInput").ap()
    wfc_d = nc.dram_tensor("wfc", [128, KH, VS], BF16, kind="ExternalInput").ap()
    bfc_d = nc.dram_tensor("bfc", [128, VS], F32, kind="ExternalInput").ap()
    xt_d = nc.dram_tensor("xt", [T - 1, 128, KX, B], BF16, kind="ExternalInput").ap()
    ht0_d = nc.dram_tensor("ht0", [128, KH, B], BF16, kind="ExternalInput").ap()
    c0_d = nc.dram_tensor("c0", [128, B], F32, kind="ExternalInput").ap()
    out_d = nc.dram_tensor("logits", [T, B, VS], BF16, kind="ExternalOutput").ap()
    hch_d = nc.dram_tensor("hch", [2, 128, B], BF16, kind="Internal").ap()
    hall_d = nc.dram_tensor("hall", [2, KH, 128, B], BF16, kind="Internal",
                            addr_space="Shared").ap()
    warm_in_d = nc.dram_tensor("warm_in", [1, 4], BF16, kind="Internal").ap()
    warm_out_d = nc.dram_tensor("warm_out", [KH, 4], BF16, kind="Internal",
                                addr_space="Shared").ap()

    # (instruction, sem, value) to inject after Tile scheduling
    pending_waits = []

    with tile.TileContext(nc) as tc:
        with (
            tc.tile_pool(name="weights", bufs=1) as wpool,
            tc.tile_pool(name="xin", bufs=3) as xpool,
            tc.tile_pool(name="gact", bufs=2) as gpool,
            tc.tile_pool(name="state", bufs=1) as spool,
            tc.tile_pool(name="hbuf", bufs=1) as hpool,
            tc.tile_pool(name="lout", bufs=4) as lpool,
            tc.tile_pool(name="pg", bufs=4, space="PSUM") as pgpool,
            tc.tile_pool(name="pf", bufs=3, space="PSUM") as pfpool,
        ):

            wih = wpool.tile([128, KX, 4, 128], BF16)
            whh = wpool.tile([128, KH, 4, 128], BF16)
            bg = wpool.tile([128, 4], F32)
            wfc = wpool.tile([128, KH, VS], BF16)
            bfc = wpool.tile([128, VS], F32)
            ht0 = hpool.tile([128, KH, B], BF16)
            c = spool.tile([128, B], F32)
            # recv ring: slot s chunk k (from core k) at recv[s][:, k, :]
            recv = [hpool.tile([128, KH, B], BF16, name=f"recv{s_}")
                    for s_ in range(NSLOT)]
            hsend = [hpool.tile([128, B], BF16, name=f"hsend{s_}")
                    for s_ in range(2)]
            tanh_c = spool.tile([128, B], F32)
            ig = spool.tile([128, B], F32)

            # warmup collective: absorb the one-time ncfw init/barrier cost
            # while the weight DMAs stream in
            nc.gpsimd.collective_compute(
                "AllGather", mybir.AluOpType.bypass,
                replica_groups=[list(range(NCORES))],
                ins=[warm_in_d], outs=[warm_out_d],
            )
            # prologue loads, consumer-ordered (sync queue): step-1 gate
            # deps first, then FC weights per vocab chunk.
            nc.sync.dma_start(ht0[:], ht0_d[:])
            nc.sync.dma_start(c[:], c0_d[:])
            xt1 = xpool.tile([128, KX, B], BF16, tag="xt")
            nc.sync.dma_start(xt1[:], xt_d[0])
            nc.sync.dma_start(wih[:], wih_d[:])
            nc.sync.dma_start(bg[:], bg_d[:])
            nc.sync.dma_start(whh[:], whh_d[:])
            for v in range(NVC):
                vsl = slice(v * VC, (v + 1) * VC)
                nc.sync.dma_start(wfc[:, :, vsl], wfc_d[:, :, vsl])
            nc.sync.dma_start(bfc[:], bfc_d[:])


            def hsrc(t):
                if t == 0:
                    return [ht0[:, k, :] for k in range(KH)]
                r = recv[t % NSLOT]
                return [r[:, k, :] for k in range(KH)]

            def emit_fc(t):
                src = hsrc(t)
                first = None
                for v in range(NVC):
                    vsl = slice(v * VC, (v + 1) * VC)
                    pf = pfpool.tile([B, VC], F32, tag="pf")
                    for k in range(KH):
                        mm = nc.tensor.matmul(
                            pf[:], src[k], wfc[:, k, vsl],
                            start=(k == 0), stop=(k == KH - 1),
                        )
                        if first is None:
                            first = mm
                    lo = lpool.tile([B, VC], BF16, tag="lo")
                    nc.vector.tensor_add(lo[:], pf[:], bfc[:, vsl])
                    nc.scalar.dma_start(out_d[t, :, vsl], lo[:])
                return first

            for t in range(1, nsteps + 1):
                if t == 1:
                    xt = xt1
                else:
                    xt = xpool.tile([128, KX, B], BF16, tag="xt")
                    nc.scalar.dma_start(xt[:], xt_d[t - 1])
                src_prev = hsrc(t - 1)

                # ---- gates (transposed): 4 col-chunks [i, g, f, o] ----
                # x-parts first (no cross-core dep: PE busy while h arrives)
                pgs = []
                for cc in range(4):
                    pg = pgpool.tile([128, 512], F32, tag="pg")
                    pgs.append(pg)
                    for k in range(KX):
                        nc.tensor.matmul(
                            pg[:, 0:B], wih[:, k, cc, :], xt[:, k, :],
                            start=(k == 0), stop=False,
                        )
                gact = []
                for cc in range(4):
                    pg = pgs[cc]
                    for k in range(KH):
                        mm = nc.tensor.matmul(
                            pg[:, 0:B], whh[:, k, cc, :], src_prev[k],
                            start=False, stop=(k == KH - 1),
                        )
                    a = gpool.tile([128, B], F32, tag=f"g{cc}")
                    nc.scalar.activation(a[:], pg[:, 0:B], _CC_FUNC[cc],
                                         bias=bg[:, cc:cc + 1])
                    gact.append(a)
                    if cc == 1:
                        nc.vector.tensor_mul(ig[:], gact[0][:], gact[1][:])
                    elif cc == 2:
                        nc.vector.tensor_mul(c[:], c[:], gact[2][:])
                        nc.vector.tensor_add(c[:], c[:], ig[:])
                        nc.scalar.activation(tanh_c[:], c[:], AF.Tanh)
                    elif cc == 3:
                        hs = hsend[t % 2]
                        nc.vector.tensor_mul(hs[:], gact[3][:], tanh_c[:])

                # ---- all-gather h_t via ncfw collective (HBM bounce) ----
                par = t % 2
                nc.sync.dma_start(hch_d[par], hsend[t % 2][:])
                nc.gpsimd.collective_compute(
                    "AllGather", mybir.AluOpType.bypass,
                    replica_groups=[list(range(NCORES))],
                    ins=[hch_d[par]], outs=[hall_d[par]],
                )
                r = recv[t % NSLOT]
                for k in range(KH):
                    nc.sync.dma_start(r[:, k, :], hall_d[par, k])

                # ---- FC of previous step: PE work overlapping the bcast ----
                emit_fc(t - 1)

            emit_fc(nsteps)

    if for_sim:
        # interp needs concrete registers + library loads, not ISA packing
        nc.dce_regs()
        nc.alloc_regs()
        nc.insert_library_loads()
        nc.insert_act_table_loads()
    else:
        nc.compile()
    return nc


_NC_CACHE = None


def _get_nc():
    global _NC_CACHE
    if _NC_CACHE is None:
        _NC_CACHE = build_nc()
    return _NC_CACHE


def _prep_inputs(encoder_output, captions, embed_table, W_ih, W_hh, b_ih, b_hh,
                 W_fc, b_fc):
    bf = ml_dtypes.bfloat16
    enc = np.asarray(encoder_output, np.float32)
    cap = np.asarray(captions).astype(np.int64)
    emb = np.asarray(embed_table, np.float32)
    W_ih = np.asarray(W_ih, np.float32)
    W_hh = np.asarray(W_hh, np.float32)
    W_fc = np.asarray(W_fc, np.float32)
    bgs = np.asarray(b_ih, np.float32) + np.asarray(b_hh, np.float32)
    b_fc = np.asarray(b_fc, np.float32)

    X = np.empty((T, B, E), np.float32)
    X[0] = enc
    X[1:] = emb[cap[:, : T - 1]].transpose(1, 0, 2)
    # xt[t,p,k,b] = X[t+1,b,k*128+p], steps 1..23
    xt = np.ascontiguousarray(
        X[1:].reshape(T - 1, B, KX, 128).transpose(0, 3, 2, 1)).astype(bf)

    # step 0 on host, fp32 (h_prev = c_prev = 0)
    gates0 = enc @ W_ih.T + bgs
    i0, f0, g0, o0 = np.split(gates0, 4, axis=-1)
    sig = lambda z: 1.0 / (1.0 + np.exp(-z))
    c0 = sig(i0) * np.tanh(g0)
    h0 = sig(o0) * np.tanh(c0)
    ht0 = np.ascontiguousarray(
        h0.T.reshape(KH, 128, B).transpose(1, 0, 2)).astype(bf)

    common = {"xt": xt, "ht0": ht0}
    in_maps = []
    for ci in range(NCORES):
        r = np.r_[ci * 128:(ci + 1) * 128]
        sel = np.concatenate([r, 2048 + r, 1024 + r, 3072 + r])  # [i,g,f,o]
        wih = np.ascontiguousarray(
            W_ih[sel].reshape(4, 128, KX, 128).transpose(3, 2, 0, 1)).astype(bf)
        whh = np.ascontiguousarray(
            W_hh[sel].reshape(4, 128, KH, 128).transpose(3, 2, 0, 1)).astype(bf)
        bg = np.ascontiguousarray(bgs[sel].reshape(4, 128).T)
        c0j = np.ascontiguousarray(c0[:, ci * 128:(ci + 1) * 128].T)
        vsl = slice(ci * VS, (ci + 1) * VS)
        wfc = np.ascontiguousarray(
            W_fc[vsl].reshape(VS, KH, 128).transpose(2, 1, 0)).astype(bf)
        bfc = np.ascontiguousarray(np.broadcast_to(b_fc[vsl], (128, VS)))
        in_maps.append({**common, "wih": wih, "whh": whh, "bg": bg,
                        "c0": c0j, "wfc": wfc, "bfc": bfc})
    return in_maps


def run_on_device(in_maps, trace=False, **kw):
    nc = _get_nc()
    return run_bass_kernel_spmd(
        nc, in_maps, list(range(NCORES)), trace=trace, **kw)


def _assemble(res):
    shards = [np.asarray(res.results[ci]["logits"]).astype(np.float32)
              for ci in range(NCORES)]
    full = np.concatenate(shards, axis=-1)  # [T, B, V]
    return np.ascontiguousarray(full.transpose(1, 0, 2))  # [B, T, V]


def kernel(encoder_output, captions, embed_table, W_ih, W_hh, b_ih, b_hh,
           W_fc, b_fc):
    in_maps = _prep_inputs(encoder_output, captions, embed_table,
                           W_ih, W_hh, b_ih, b_hh, W_fc, b_fc)
    res = run_on_device(in_maps)
    return _assemble(res)


# revision 20
# speedup vs baseline: 1.1933x; 1.1574x over previous
"""LSTM caption decoder on 8 TRN2 NeuronCores — hidden-sharded.

Problem: 24-step LSTMCell (B=128, E=512, H=1024) + vocab projection (V=12000).

Strategy (vs the replicated-LSTM baseline): shard the LSTM hidden dim 8-way.
Core j owns hidden columns j*128..(j+1)*128, i.e. 512 of the 4096 gate
columns (128 each of i,g,f,o). Per step each core computes only its gate
slice (contraction over the FULL h), updates its c/h column block, and
all-gathers its h_t chunk [128, B] to the other cores with an ncfw
collective AllGather (SBUF -> HBM -> AllGather -> HBM -> SBUF), bounced
through a triple-buffered recv ring. The vocab projection stays
column-sharded (1500 cols/core) and is interleaved into the step loop so
the PE computes FC_{t-1} while the h_t all-gather is in flight.

(A faster SBUF->SBUF remote_dma_broadcast path exists and validates in the
8-core interpreter, but issuing more than one data broadcast crashes the
exec unit under this runtime — see kernel_rdma.py; sem-only broadcasts
repeat fine. The ncfw collective is the supported path; its ~9us Mesh
latency + ~25us one-time barrier dominate the remaining runtime.)

Layouts: gates are computed TRANSPOSED — out[gate_cols(128), B] with the
weight block stationary — so h_t is produced directly in [cols, B] chunk
layout and no transposes are needed anywhere. Gate bias rides the ACT op
(per-partition bias in transposed layout).

Per-core PE work/step: 48 gate matmuls (12 K-chunks x 4 col-chunks, 128
moving rows) + 24 FC matmuls (8 K-chunks x 3 vocab chunks, 500 rows)
= 6144 + 12000 cycles ~ 7.6us at 2.4GHz. Measured step period ~24.5us
(AG-latency bound): 635us total vs 651us for the replicated baseline.

All matmul inputs bf16 (fp32 PSUM accumulation); c state fp32; logits
stored bf16 (adds ~0.1% rms; tolerance is 2e-2). End-to-end rel err vs
the fp32 reference: 2.96e-3.
"""

import sys

if "/opt/trn_rl_repo" not in sys.path:
    sys.path.insert(0, "/opt/trn_rl_repo")

import numpy as np
import ml_dtypes

import concourse.bass as bass
import concourse.bacc as bacc
import concourse.tile as tile
from concourse import mybir
from concourse.bass import ds
from concourse.bass_utils import run_bass_kernel_spmd

B = 128
T = 24
E = 512
H = 1024
V = 12000
NCORES = 8
VS = V // NCORES          # 1500 vocab cols per core
KX = E // 128             # 4 contraction chunks for x-part
KH = H // 128             # 8 contraction chunks for h-part
NVC = 3                   # vocab chunks per core (3 x 500)
VC = VS // NVC            # 500
NSLOT = 3                 # recv ring depth for h chunks
TRIG_WAITS = True         # debug toggle
SKIP_COMMS = False        # debug toggle: no broadcasts/triggers/waits

F32 = mybir.dt.float32
BF16 = mybir.dt.bfloat16
AF = mybir.ActivationFunctionType
POOL = mybir.EngineType.Pool

# gate col-chunk order [i, g, f, o] -> activation per chunk
_CC_FUNC = [AF.Sigmoid, AF.Tanh, AF.Sigmoid, AF.Sigmoid]


def build_nc(for_sim: bool = False, nsteps: int = T - 1):
    """nsteps < T-1 builds a truncated kernel (debug bisection only)."""
    nc = bacc.Bacc("TRN2", target_bir_lowering=False, debug=False,
                   num_devices=NCORES)

    wih_d = nc.dram_tensor("wih", [128, KX, 4, 128], BF16, kind="ExternalInput").ap()
    whh_d = nc.dram_tensor("whh", [128, KH, 4, 128], BF16, kind="ExternalInput").ap()
    bg_d = nc.dram_tensor("bg", [128, 4], F32, kind="ExternalInput").ap()
    wfc_d = nc.dram_tensor("wfc", [128, KH, VS], BF16, kind="ExternalInput").ap()
    bfc_d = nc.dram_tensor("bfc", [128, VS], F32, kind="ExternalInput").ap()
    xt_d = nc.dram_tensor("xt", [T - 1, 128, KX, B], BF16, kind="ExternalInput").ap()
    ht0_d = nc.dram_tensor("ht0", [128, KH, B], BF16, kind="ExternalInput").ap()
    c0_d = nc.dram_tensor("c0", [128, B], F32, kind="ExternalInput").ap()
    out_d = nc.dram_tensor("logits", [T, B, VS], BF16, kind="ExternalOutput").ap()
    hch_d = nc.dram_tensor("hch", [2, 128, B], BF16, kind="Internal").ap()
    hall_d = nc.dram_tensor("hall", [2, KH, 128, B], BF16, kind="Internal",
                            addr_space="Shared").ap()
    warm_in_d = nc.dram_tensor("warm_in", [1, 4], BF16, kind="Internal").ap()
    warm_out_d = nc.dram_tensor("warm_out", [KH, 4], BF16, kind="Internal",
                                addr_space="Shared").ap()

    # (instruction, sem, value) to inject after Tile scheduling
    pending_waits = []

    with tile.TileContext(nc) as tc:
        with (
            tc.tile_pool(name="weights", bufs=1) as wpool,
            tc.tile_pool(name="xin", bufs=3) as xpool,
            tc.tile_pool(name="gact", bufs=2) as gpool,
            tc.tile_pool(name="state", bufs=1) as spool,
            tc.tile_pool(name="hbuf", bufs=1) as hpool,
            tc.tile_pool(name="lout", bufs=4) as lpool,
            tc.tile_pool(name="pg", bufs=4, space="PSUM") as pgpool,
            tc.tile_pool(name="pf", bufs=3, space="PSUM") as pfpool,
        ):

            wih = wpool.tile([128, KX, 4, 128], BF16)
            whh = wpool.tile([128, KH, 4, 128], BF16)
            bg = wpool.tile([128, 4], F32)
            wfc = wpool.tile([128, KH, VS], BF16)
            bfc = wpool.tile([128, VS], F32)
            ht0 = hpool.tile([128, KH, B], BF16)
            c = spool.tile([128, B], F32)
            # recv ring: slot s chunk k (from core k) at recv[s][:, k, :]
            recv = [hpool.tile([128, KH, B], BF16, name=f"recv{s_}")
                    for s_ in range(NSLOT)]
            hsend = [hpool.tile([128, B], BF16, name=f"hsend{s_}")
                    for s_ in range(2)]
            tanh_c = spool.tile([128, B], F32)
            ig = spool.tile([128, B], F32)

            # warmup collective: absorb the one-time ncfw init/barrier cost
            # while the weight DMAs stream in
            nc.gpsimd.collective_compute(
                "AllGather", mybir.AluOpType.bypass,
                replica_groups=[list(range(NCORES))],
                ins=[warm_in_d], outs=[warm_out_d],
            )
            # prologue loads, consumer-ordered (sync queue): step-1 gate
            # deps first, then FC weights per vocab chunk.
            nc.sync.dma_start(ht0[:], ht0_d[:])
            nc.sync.dma_start(c[:], c0_d[:])
            xt1 = xpool.tile([128, KX, B], BF16, tag="xt")
            nc.sync.dma_start(xt1[:], xt_d[0])
            nc.sync.dma_start(wih[:], wih_d[:])
            nc.sync.dma_start(bg[:], bg_d[:])
            nc.sync.dma_start(whh[:], whh_d[:])
            for v in range(NVC):
                vsl = slice(v * VC, (v + 1) * VC)
                nc.sync.dma_start(wfc[:, :, vsl], wfc_d[:, :, vsl])
            nc.sync.dma_start(bfc[:], bfc_d[:])


            def hsrc(t):
                if t == 0:
                    return [ht0[:, k, :] for k in range(KH)]
                r = recv[t % NSLOT]
                return [r[:, k, :] for k in range(KH)]

            def emit_fc(t):
                src = hsrc(t)
                first = None
                for v in range(NVC):
                    vsl = slice(v * VC, (v + 1) * VC)
                    pf = pfpool.tile([B, VC], F32, tag="pf")
                    for k in range(KH):
                        mm = nc.tensor.matmul(
                            pf[:], src[k], wfc[:, k, vsl],
                            start=(k == 0), stop=(k == KH - 1),
                        )
                        if first is None:
                            first = mm
                    lo = lpool.tile([B, VC], BF16, tag="lo")
                    nc.vector.tensor_add(lo[:], pf[:], bfc[:, vsl])
                    nc.scalar.dma_start(out_d[t, :, vsl], lo[:])
                return first

            for t in range(1, nsteps + 1):
                if t == 1:
                    xt = xt1
                else:
                    xt = xpool.tile([128, KX, B], BF16, tag="xt")
                    nc.scalar.dma_start(xt[:], xt_d[t - 1])
                src_prev = hsrc(t - 1)

                # ---- gates (transposed): 4 col-chunks [i, g, f, o] ----
                # x-parts first (no cross-core dep: PE busy while h arrives)
                pgs = []
                for cc in range(4):
                    pg = pgpool.tile([128, 512], F32, tag="pg")
                    pgs.append(pg)
                    for k in range(KX):
                        nc.tensor.matmul(
                            pg[:, 0:B], wih[:, k, cc, :], xt[:, k, :],
                            start=(k == 0), stop=False,
                        )
                gact = []
                for cc in range(4):
                    pg = pgs[cc]
                    for k in range(KH):
                        mm = nc.tensor.matmul(
                            pg[:, 0:B], whh[:, k, cc, :], src_prev[k],
                            start=False, stop=(k == KH - 1),
                        )
                    a = gpool.tile([128, B], F32, tag=f"g{cc}")
                    nc.scalar.activation(a[:], pg[:, 0:B], _CC_FUNC[cc],
                                         bias=bg[:, cc:cc + 1])
                    gact.append(a)
                    if cc == 1:
                        nc.vector.tensor_mul(ig[:], gact[0][:], gact[1][:])
                    elif cc == 2:
                        nc.vector.tensor_mul(c[:], c[:], gact[2][:])
                        nc.vector.tensor_add(c[:], c[:], ig[:])
                        nc.scalar.activation(tanh_c[:], c[:], AF.Tanh)
                    elif cc == 3:
                        hs = hsend[t % 2]
                        nc.vector.tensor_mul(hs[:], gact[3][:], tanh_c[:])

                # ---- all-gather h_t via ncfw collective (HBM bounce) ----
                par = t % 2
                nc.sync.dma_start(hch_d[par], hsend[t % 2][:])
                nc.gpsimd.collective_compute(
                    "AllGather", mybir.AluOpType.bypass,
                    replica_groups=[list(range(NCORES))],
                    ins=[hch_d[par]], outs=[hall_d[par]],
                )
                # one DMA for all 8 chunks: permuted DRAM AP
                # hall[par] is [KH,128,B]; out partition p reads (k,b) at
                # k*128*B + p*B + b
                r = recv[t % NSLOT]
                src = bass.AP(hall_d.tensor, par * KH * 128 * B,
                              [[B, 128], [128 * B, KH], [1, B]])
                nc.sync.dma_start(r[:], src)

                # ---- FC of previous step: PE work overlapping the bcast ----
                emit_fc(t - 1)

            emit_fc(nsteps)

    if for_sim:
        # interp needs concrete registers + library loads, not ISA packing
        nc.dce_regs()
        nc.alloc_regs()
        nc.insert_library_loads()
        nc.insert_act_table_loads()
    else:
        nc.compile()
    return nc


_NC_CACHE = None


def _get_nc():
    global _NC_CACHE
    if _NC_CACHE is None:
        _NC_CACHE = build_nc()
    return _NC_CACHE


def _prep_inputs(encoder_output, captions, embed_table, W_ih, W_hh, b_ih, b_hh,
                 W_fc, b_fc):
    bf = ml_dtypes.bfloat16
    enc = np.asarray(encoder_output, np.float32)
    cap = np.asarray(captions).astype(np.int64)
    emb = np.asarray(embed_table, np.float32)
    W_ih = np.asarray(W_ih, np.float32)
    W_hh = np.asarray(W_hh, np.float32)
    W_fc = np.asarray(W_fc, np.float32)
    bgs = np.asarray(b_ih, np.float32) + np.asarray(b_hh, np.float32)
    b_fc = np.asarray(b_fc, np.float32)

    X = np.empty((T, B, E), np.float32)
    X[0] = enc
    X[1:] = emb[cap[:, : T - 1]].transpose(1, 0, 2)
    # xt[t,p,k,b] = X[t+1,b,k*128+p], steps 1..23
    xt = np.ascontiguousarray(
        X[1:].reshape(T - 1, B, KX, 128).transpose(0, 3, 2, 1)).astype(bf)

    # step 0 on host, fp32 (h_prev = c_prev = 0)
    gates0 = enc @ W_ih.T + bgs
    i0, f0, g0, o0 = np.split(gates0, 4, axis=-1)
    sig = lambda z: 1.0 / (1.0 + np.exp(-z))
    c0 = sig(i0) * np.tanh(g0)
    h0 = sig(o0) * np.tanh(c0)
    ht0 = np.ascontiguousarray(
        h0.T.reshape(KH, 128, B).transpose(1, 0, 2)).astype(bf)

    common = {"xt": xt, "ht0": ht0}
    in_maps = []
    for ci in range(NCORES):
        r = np.r_[ci * 128:(ci + 1) * 128]
        sel = np.concatenate([r, 2048 + r, 1024 + r, 3072 + r])  # [i,g,f,o]
        wih = np.ascontiguousarray(
            W_ih[sel].reshape(4, 128, KX, 128).transpose(3, 2, 0, 1)).astype(bf)
        whh = np.ascontiguousarray(
            W_hh[sel].reshape(4, 128, KH, 128).transpose(3, 2, 0, 1)).astype(bf)
        bg = np.ascontiguousarray(bgs[sel].reshape(4, 128).T)
        c0j = np.ascontiguousarray(c0[:, ci * 128:(ci + 1) * 128].T)
        vsl = slice(ci * VS, (ci + 1) * VS)
        wfc = np.ascontiguousarray(
            W_fc[vsl].reshape(VS, KH, 128).transpose(2, 1, 0)).astype(bf)
        bfc = np.ascontiguousarray(np.broadcast_to(b_fc[vsl], (128, VS)))
        in_maps.append({**common, "wih": wih, "whh": whh, "bg": bg,
                        "c0": c0j, "wfc": wfc, "bfc": bfc})
    return in_maps


def run_on_device(in_maps, trace=False, **kw):
    nc = _get_nc()
    return run_bass_kernel_spmd(
        nc, in_maps, list(range(NCORES)), trace=trace, **kw)


def _assemble(res):
    shards = [np.asarray(res.results[ci]["logits"]).astype(np.float32)
              for ci in range(NCORES)]
    full = np.concatenate(shards, axis=-1)  # [T, B, V]
    return np.ascontiguousarray(full.transpose(1, 0, 2))  # [B, T, V]


def kernel(encoder_output, captions, embed_table, W_ih, W_hh, b_ih, b_hh,
           W_fc, b_fc):
    in_maps = _prep_inputs(encoder_output, captions, embed_table,
                           W_ih, W_hh, b_ih, b_hh, W_fc, b_fc)
    res = run_on_device(in_maps)
    return _assemble(res)
